# revision 1
# baseline (speedup 1.0000x reference)
"""nn_Detection_CrossEntropy Trainium2 kernel (8 NeuronCores, pure data parallel).

Each core processes one sample b of output[B=8, N=25200, 85] end to end, in
row-windows of [25,50,50,50,25] (128 partitions x W rows; small first/last
window shortens pipeline fill/drain):
  mask[g,n] = [IoU(gt_g, pred_n) >= 0.5]  (computed as 3*inter >= parea+garea,
              via a min/add chain split across DVE and GPSIMD)
  one PSUM-accumulated PE matmul per row: T += mask^T @ [obj*logits | LSE | 1]
  loss_b = (sum(T[:,80]) - sum_g T[g, cls_g]) / sum(T[:,81])
Engine split: DVE (min/cmp/reduce), GPSIMD (obj premult + width adds),
ACT (exp/ln), PE (matmuls). Host only pads/reshapes and gathers [32,82].
"""
import numpy as np

"""Workaround: this container's walrus rejects >2 sync waits on the
TileContext tail Drain (setupSyncWait<CTRL_NO_STRUCT>: "Too many sync
wait commands"). Split the tail-drain waits across multiple drains."""
import concourse.mybir as mybir
from concourse import tile
from concourse.vector_clock import ScopedClock

MAXW = 1

def _drain_and_barrier(self, tick_clock, wait_clock):
    nc = self.nc
    drain_inst = nc.sync.drain()
    wait_clock.add_sem_waits(drain_inst.ins, ScopedClock({None: tick_clock.global_clock}))
    si = drain_inst.ins.sync_info
    if si is not None and si.on_wait is not None and len(si.on_wait) > MAXW:
        waits = list(si.on_wait)
        si.on_wait = waits[:MAXW]
        for i in range(MAXW, len(waits), MAXW):
            extra = nc.sync.drain()
            esi = extra.ins.sync_info
            if esi is None:
                extra.ins.sync_info = mybir.SyncInfo(on_wait=waits[i:i+MAXW], on_update=[])
            else:
                esi.on_wait = waits[i:i+MAXW]
    nc.all_engine_barrier()
    assert self.sems is not None
    popped = nc._tile_sem_poison_stack.pop()
    assert popped is self._sem_poison
    nc.clear_and_free_semaphores(list(self.sems.allocated().values()))
    nc.all_engine_barrier()

tile.TileContext._drain_and_barrier = _drain_and_barrier


# General fix: this walrus accepts at most ONE sync wait per instruction.
# Split extra waits onto preceding Drain carriers at BIR-JSON level.
import orjson
import concourse.bass as _bass

_orig_to_json_bytes = _bass.Bass.to_json_bytes

def _to_json_bytes_split(self) -> bytes:
    j = orjson.loads(_orig_to_json_bytes(self))
    for f in j.get("functions", []):
        for bb in f.get("blocks", []):
            out = []
            changed = False
            for i in bb.get("instructions", []):
                si = i.get("sync_info")
                ow = (si or {}).get("on_wait") or []
                if len(ow) > 1:
                    changed = True
                    for k, w in enumerate(ow[:-1]):
                        out.append({
                            "name": f'{i["name"]}-w{k}',
                            "opcode": "Drain",
                            "engine": i["engine"],
                            "ins": [],
                            "outs": [],
                            "debug": i.get("debug", 0),
                            "sync_info": {"on_update": [], "on_wait": [w]},
                        })
                    si["on_wait"] = [ow[-1]]
                out.append(i)
            if changed:
                bb["instructions"] = out
    return orjson.dumps(j)

_bass.Bass.to_json_bytes = _to_json_bytes_split


# Custom fused DVE op: out = relu((Src0 - Src1) * imm2)
import numpy as _np
from concourse.dve_spec import Spec as _Spec, Src0 as _S0, Src1 as _S1, C2 as _C2, relu as _relu
from concourse import dve_ops as _dve_ops

RELU_SUB_SCALE_ANT = _dve_ops.DveOp(
    "RELU_SUB_SCALE_ANT",
    _Spec(
        body=_relu((_S0 - _S1) * _C2),
        reference=lambda in0, in1, s0, s1, imm2: _np.maximum(
            (in0.astype(_np.float32) - in1) * imm2, 0.0
        ).astype(_np.float32),
    ),
    subdim=False,
    uops_sha={"v3": "32e47ef44d8a40e4", "v4": "9aa82df2ee6912e4"},
)
_dve_ops.OPS.append(RELU_SUB_SCALE_ANT)
_dve_ops.CUSTOM_DVE_SPECS[RELU_SUB_SCALE_ANT.name] = RELU_SUB_SCALE_ANT.spec
_dve_ops._SUB_OPCODE_FOR_NAME[RELU_SUB_SCALE_ANT.name] = 17



# kernel builder:


import numpy as np
import concourse.bass as bass
import concourse.mybir as mybir
from concourse import tile

F32 = mybir.dt.float32
ALU = mybir.AluOpType
ACTF = mybir.ActivationFunctionType

N, G, C = 25200, 32, 80
NPAD = 25600
P = 128
R = NPAD // P            # 200 rows per partition
ROW = 85
SCALE = 640.0
WINDOWS = [25, 50, 50, 50, 25]   # small first/last window: cut fill/drain


def build_kernel(outer=1, row_exp=False, gps_pair=False, gps_premult=False, use_custom=False):
    nc = bass.Bass()
    data = nc.declare_dram_parameter("data", [P, R * ROW], F32, isOutput=False)
    lb = nc.declare_dram_parameter("lb", [G, 5], F32, isOutput=False)
    res = nc.declare_dram_parameter("res", [G, 82], F32, isOutput=True)
    gt_bounce = nc.dram_tensor("gt_bounce", [G * 5], F32)

    with tile.TileContext(nc) as tc:
        with (
            tc.tile_pool(name="const", bufs=1) as constp,
            tc.tile_pool(name="main", bufs=2) as mainp,
            tc.tile_pool(name="sc", bufs=2) as scp,
            tc.tile_pool(name="cols", bufs=2) as colsp,
            tc.tile_pool(name="pair", bufs=2) as pairp,
            tc.tile_pool(name="psum", bufs=1, space="PSUM") as psump,
        ):
          for _o in range(outer):
            # ---------------- GT prep (once) ----------------
            lbt = constp.tile([G, 5], F32, name="lbt")
            nc.sync.dma_start(lbt[:], lb[:, :])
            gx, gy = lbt[:, 1:2], lbt[:, 2:3]
            gw, gh = lbt[:, 3:4], lbt[:, 4:5]
            pack = constp.tile([G, 5], F32, name="pack")  # gx1,gx2,gy1,gy2,ga
            raw = constp.tile([G, 4], F32, name="raw")
            nc.vector.scalar_tensor_tensor(raw[:, 0:1], gw, -0.5, gx, ALU.mult, ALU.add)
            nc.vector.scalar_tensor_tensor(raw[:, 1:2], gw, 0.5, gx, ALU.mult, ALU.add)
            nc.vector.scalar_tensor_tensor(raw[:, 2:3], gh, -0.5, gy, ALU.mult, ALU.add)
            nc.vector.scalar_tensor_tensor(raw[:, 3:4], gh, 0.5, gy, ALU.mult, ALU.add)
            clp = constp.tile([G, 4], F32, name="clp")
            nc.vector.tensor_scalar(clp[:], raw[:], 0.0, 1.0, ALU.max, ALU.min)
            nc.vector.tensor_scalar_mul(pack[:, 0:4], clp[:], SCALE)
            wt = constp.tile([G, 2], F32, name="wt")
            nc.vector.tensor_sub(wt[:, 0:1], pack[:, 1:2], pack[:, 0:1])
            nc.vector.tensor_sub(wt[:, 1:2], pack[:, 3:4], pack[:, 2:3])
            nc.vector.tensor_mul(pack[:, 4:5], wt[:, 0:1], wt[:, 1:2])
            # negate gx1, gy1 in place (packed cols 0, 2) for the add-form chain
            nc.vector.tensor_scalar_mul(pack[:, 0:1], pack[:, 0:1], -1.0)
            nc.vector.tensor_scalar_mul(pack[:, 2:3], pack[:, 2:3], -1.0)
            nc.sync.dma_start(gt_bounce[:].rearrange("(q g) -> g q", g=G), pack[:])
            gt_bc = constp.tile([P, 5 * G], F32, name="gt_bc")
            nc.sync.dma_start(gt_bc[:], gt_bounce[:][None, :].partition_broadcast(P))
            gx1_t = gt_bc[:, 0 * G : 1 * G]
            gx2_t = gt_bc[:, 1 * G : 2 * G]
            gy1_t = gt_bc[:, 2 * G : 3 * G]
            gy2_t = gt_bc[:, 3 * G : 4 * G]
            ga_t = gt_bc[:, 4 * G : 5 * G]

            psum_T = psump.tile([G, 82], F32, name="psum_T")

            r_base = 0
            for w, W in enumerate(WINDOWS):
                mt = mainp.tile([P, W * ROW], F32, tag="mt", name="mt")
                nc.sync.dma_start(
                    mt[:], data[:, r_base * ROW : (r_base + W) * ROW]
                )
                m3 = mt[:].rearrange("p (r c) -> p r c", c=ROW)
                x_c, y_c = m3[:, :, 0], m3[:, :, 1]
                w_c, h_c = m3[:, :, 2], m3[:, :, 3]
                obj_c = m3[:, :, 4]

                cols = colsp.tile([P, W * 5], F32, tag="cols", name="cols")
                c3 = cols[:].rearrange("p (q r) -> p q r", q=5)
                px1, px2 = c3[:, 0, :], c3[:, 1, :]
                py1, py2 = c3[:, 2, :], c3[:, 3, :]
                parea = c3[:, 4, :]
                nc.vector.scalar_tensor_tensor(px1, w_c, 0.5, x_c, ALU.mult, ALU.subtract)
                nc.vector.scalar_tensor_tensor(px2, w_c, 0.5, x_c, ALU.mult, ALU.add)
                nc.vector.scalar_tensor_tensor(py1, h_c, 0.5, y_c, ALU.mult, ALU.subtract)
                nc.vector.scalar_tensor_tensor(py2, h_c, 0.5, y_c, ALU.mult, ALU.add)
                nc.vector.tensor_mul(parea, w_c, h_c)

                # ---- scaled logits + LSE ----
                scaled = scp.tile([P, W * 82], F32, tag="scaled", name="scaled")
                s3 = scaled[:].rearrange("p (r c) -> p r c", c=82)
                sums = colsp.tile([P, W], F32, tag="sums", name="sums")
                ob = obj_c[:, :, None].broadcast_to([P, W, C])
                pm_eng = nc.gpsimd if gps_premult else nc.vector
                pm_eng.tensor_tensor(s3[:, :, 0:C], m3[:, :, 5:ROW], ob, ALU.mult)
                if row_exp:
                    scr = scp.tile([P, C], F32, tag="scr", name="scr")
                    for rr in range(W):
                        nc.scalar.activation(
                            scr[:], s3[:, rr, 0:C], ACTF.Exp,
                            accum_out=sums[:, rr : rr + 1],
                        )
                else:
                    expt = scp.tile([P, W * C], F32, tag="expt", name="expt")
                    nc.scalar.activation(expt[:], s3[:, :, 0:C], ACTF.Exp)
                    nc.vector.tensor_reduce(
                        sums[:],
                        expt[:].rearrange("p (r c) -> p r c", c=C),
                        mybir.AxisListType.X, ALU.add,
                    )
                lsew = colsp.tile([P, W], F32, tag="lsew", name="lsew")
                nc.scalar.activation(lsew[:], sums[:], ACTF.Ln)
                nc.vector.tensor_copy(s3[:, :, 80], lsew[:])
                nc.gpsimd.memset(s3[:, :, 81], 1.0)

                # ---- IoU mask ----
                def pb(col):
                    return col[:, :, None].broadcast_to([P, W, G])
                def gb(t):
                    return t[:, None, :].broadcast_to([P, W, G])
                sh = lambda t: t[:].rearrange("p (r g) -> p r g", g=G)

                A = pairp.tile([P, W * G], F32, tag="A", name="A")
                B = pairp.tile([P, W * G], F32, tag="B", name="B")
                Cc = pairp.tile([P, W * G], F32, tag="Cc", name="Cc")
                D = pairp.tile([P, W * G], F32, tag="D", name="D")
                GP = pairp.tile([P, W * G], F32, tag="GP", name="GP")
                # A = -max(px1,gx1) = min(px1n, gx1n); same for Cc (y)
                nc.vector.tensor_tensor(sh(A), pb(px1), gb(gx1_t), ALU.min)
                nc.vector.tensor_tensor(sh(B), pb(px2), gb(gx2_t), ALU.min)
                nc.vector.tensor_tensor(sh(GP), pb(parea), gb(ga_t), ALU.add)
                nc.vector.tensor_tensor(sh(Cc), pb(py1), gb(gy1_t), ALU.min)
                nc.vector.tensor_tensor(sh(D), pb(py2), gb(gy2_t), ALU.min)
                # wx = B + A (GPS add), wy = D + Cc (GPS add)
                nc.gpsimd.tensor_tensor(B[:], B[:], A[:], ALU.add)
                nc.gpsimd.tensor_tensor(D[:], D[:], Cc[:], ALU.add)
                nc.vector.tensor_scalar(B[:], B[:], 0.0, 3.0, ALU.max, ALU.mult)
                nc.vector.tensor_scalar_max(D[:], D[:], 0.0)
                nc.vector.tensor_mul(B[:], B[:], D[:])     # V
                nc.vector.tensor_tensor(B[:], B[:], GP[:], ALU.is_ge)  # MK

                for rr in range(W):
                    r = r_base + rr
                    nc.tensor.matmul(
                        psum_T[:],
                        B[:, rr * G : (rr + 1) * G],
                        s3[:, rr, 0:82],
                        start=(r == 0),
                        stop=(r == R - 1),
                    )
                r_base += W

            out_t = constp.tile([G, 82], F32, name="out_t")
            nc.vector.tensor_copy(out_t[:], psum_T[:])
            nc.sync.dma_start(res[:, :], out_t[:])
    return nc


def host_finish(res_list, label_batch):
    B = len(res_list)
    out = np.empty((1, B), np.float32)
    for b in range(B):
        T = res_list[b]
        cls = np.asarray(label_batch)[b, :, 0].astype(np.int32)
        S_T = T[np.arange(G), cls].sum()
        S_L = T[:, 80].sum()
        S_0 = T[:, 81].sum()
        out[0, b] = (S_L - S_T) / S_0
    return out


def prep_inputs(output, label_batch):
    B = output.shape[0]
    pad = np.zeros((B, NPAD - N, ROW), output.dtype)
    data = np.concatenate([np.asarray(output), pad], axis=1)
    data = data.reshape(B, P, R * ROW)
    return [{"data": data[b], "lb": np.asarray(label_batch[b])} for b in range(B)]


_CACHE = {}


def kernel(output, label_batch, prob_threshold):
    """Full inputs -> [1, B] loss. prob_threshold == 0 for this problem
    (keep = obj >= 0 is always true; padded rows are masked geometrically)."""
    from concourse.bass_utils import run_bass_kernel_spmd

    output = np.asarray(output)
    label_batch = np.asarray(label_batch)
    B = output.shape[0]
    if "nc" not in _CACHE:
        _CACHE["nc"] = build_kernel(gps_premult=True)
    nc = _CACHE["nc"]
    in_maps = prep_inputs(output, label_batch)
    r = run_bass_kernel_spmd(nc, in_maps, list(range(B)))
    res_list = [r.results[b]["res"] for b in range(B)]
    return host_finish(res_list, label_batch).astype(output.dtype)



# revision 2
# speedup vs baseline: 2.5641x; 2.5641x over previous
"""nn_Detection_CrossEntropy Trainium2 kernel v2 (fp16 compute, 8 cores).

Each core processes one sample. Host pre-packs (per core):
  - data [128, 200*85] fp16: windowed planar layout. Per window of W rows:
    [x|y|w|h|obj planes (5*W)] + [logits c-major (80*W)], so every DVE
    operand is inner-step-1 fp16 -> 2x (tensor_tensor) / 4x (tensor_scalar)
    DVE perf modes, and matmuls run at fp16 rate (1 cyc/row vs 4 for fp32).
  - gtrep [5, 32, 50] fp16: per-gt constants (gx2, -gx1, gy2, -gy1, ga/3)
    replicated 50x so the gt side of pairwise ops is also step-1.
Pred mapping n = 128*r + p puts each row r's 128 preds across partitions,
so mask column slices feed the PE directly as matmul stationary operands.

mask[g,n] = [relu(wx)*wy >= pa/3 + ga/3], wx = min(px2,gx2) + min(-px1,-gx1)
(single relu suffices: wy<0 makes the product negative, GP>0).
T[g,:] accumulates in PSUM over 200 matmuls: T += mask^T @ [obj*logits|LSE|1].
loss_b = (sum_g T[g,80] - sum_g T[g,cls_g]) / sum_g T[g,81]   (host finish)
"""
import numpy as np

"""Workaround: this container's walrus rejects >2 sync waits on the
TileContext tail Drain (setupSyncWait<CTRL_NO_STRUCT>: "Too many sync
wait commands"). Split the tail-drain waits across multiple drains."""
import concourse.mybir as mybir
from concourse import tile
from concourse.vector_clock import ScopedClock

MAXW = 1

def _drain_and_barrier(self, tick_clock, wait_clock):
    nc = self.nc
    drain_inst = nc.sync.drain()
    wait_clock.add_sem_waits(drain_inst.ins, ScopedClock({None: tick_clock.global_clock}))
    si = drain_inst.ins.sync_info
    if si is not None and si.on_wait is not None and len(si.on_wait) > MAXW:
        waits = list(si.on_wait)
        si.on_wait = waits[:MAXW]
        for i in range(MAXW, len(waits), MAXW):
            extra = nc.sync.drain()
            esi = extra.ins.sync_info
            if esi is None:
                extra.ins.sync_info = mybir.SyncInfo(on_wait=waits[i:i+MAXW], on_update=[])
            else:
                esi.on_wait = waits[i:i+MAXW]
    nc.all_engine_barrier()
    assert self.sems is not None
    popped = nc._tile_sem_poison_stack.pop()
    assert popped is self._sem_poison
    nc.clear_and_free_semaphores(list(self.sems.allocated().values()))
    nc.all_engine_barrier()

tile.TileContext._drain_and_barrier = _drain_and_barrier


# General fix: this walrus accepts at most ONE sync wait per instruction.
# Split extra waits onto preceding Drain carriers at BIR-JSON level.
import orjson
import concourse.bass as _bass

_orig_to_json_bytes = _bass.Bass.to_json_bytes

def _to_json_bytes_split(self) -> bytes:
    j = orjson.loads(_orig_to_json_bytes(self))
    for f in j.get("functions", []):
        for bb in f.get("blocks", []):
            out = []
            changed = False
            for i in bb.get("instructions", []):
                si = i.get("sync_info")
                ow = (si or {}).get("on_wait") or []
                if len(ow) > 1:
                    changed = True
                    for k, w in enumerate(ow[:-1]):
                        out.append({
                            "name": f'{i["name"]}-w{k}',
                            "opcode": "Drain",
                            "engine": i["engine"],
                            "ins": [],
                            "outs": [],
                            "debug": i.get("debug", 0),
                            "sync_info": {"on_update": [], "on_wait": [w]},
                        })
                    si["on_wait"] = [ow[-1]]
                out.append(i)
            if changed:
                bb["instructions"] = out
    return orjson.dumps(j)

_bass.Bass.to_json_bytes = _to_json_bytes_split


# kernel builder:

import concourse.bass as bass

F32 = mybir.dt.float32
F16 = mybir.dt.float16
ALU = mybir.AluOpType
ACTF = mybir.ActivationFunctionType

N, G, C = 25200, 32, 80
P = 128
R = 198                  # ceil(25200/128)=197 -> 198 keeps windows even
NPAD = R * P             # 25344; pred n = 128*r + p
ROW = 86                 # 6 host-computed planes (px2,px1n,py2,py1n,pa3,obj) + 80 logits
SCALE = 640.0
WINDOWS = [10, 48, 48, 48, 34, 10]   # even sizes (fp16 4B-alignment of row slices)
WMAX = max(WINDOWS)


def build_kernel_v2(
    windows=None,
    prefetch=True,
    gt_two_stage=False,
    wy_pool=True,
    gpa_pool=True,
    ones_pool=True,
    tree_pool_from=2,     # tree levels >= this index run on POOL (99 = all DVE)
    premult_first=True,
):
    windows = windows or WINDOWS
    wmax = max(windows)
    nc = bass.Bass()
    data = nc.declare_dram_parameter("data", [P, R * ROW], F16, isOutput=False)
    gtrep = nc.declare_dram_parameter("gtrep", [5 * G * 2], F16, isOutput=False)
    res = nc.declare_dram_parameter("res", [G, 82], F32, isOutput=True)

    with tile.TileContext(nc) as tc:
        with (
            tc.tile_pool(name="const", bufs=1) as constp,
            tc.tile_pool(name="main", bufs=2) as mainp,
            tc.tile_pool(name="sc", bufs=2) as scp,
            tc.tile_pool(name="pair", bufs=2) as pairp,
            tc.tile_pool(name="psum", bufs=1, space="PSUM") as psump,
        ):
            # window-0 data first (unblocks premult), then the tiny gt
            # broadcast, then the remaining windows.
            # gt constants, replicated 2x: [P, 5, G, 2] (q: gx2, -gx1, gy2, -gy1, ga/3).
            # The [1,2] inner AP dim keeps pairwise ops step-1 (2x DVE mode) while
            # the broadcast DMA stays tiny (80 KB instead of 2 MB).
            gt_bc = constp.tile([P, 5 * G * 2], F16, name="gt_bc")
            mts = []
            r_base = 0
            for _w, W in enumerate(windows):
                off = ROW * r_base
                mt = mainp.tile(
                    [P, ROW * wmax], F16, tag="mt", name=f"mt{_w}",
                    bufs=(len(windows) if prefetch else 2),
                )[:, : ROW * W]
                nc.sync.dma_start(mt[:], data[:, off : off + ROW * W])
                mts.append(mt)
                r_base += W
                if _w == 0:
                    nc.sync.dma_start(
                        gt_bc[:], gtrep[:][None, :].partition_broadcast(P)
                    )
            gv = gt_bc[:].rearrange("p (q g t) -> p q g t", q=5, g=G)

            psum_T = psump.tile([G, 82], F32, name="psum_T")

            r_base = 0
            for _w, W in enumerate(windows):
                last = _w == len(windows) - 1
                mt = mts[_w]
                mv = mt[:].rearrange("p (c j) -> p c j", c=ROW)
                px2, px1n = mv[:, 0, :], mv[:, 1, :]
                py2, py1n = mv[:, 2, :], mv[:, 3, :]
                pa3, obj = mv[:, 4, :], mv[:, 5, :]
                lg = mt[:, 6 * W : ROW * W]           # [P, 80*W] c-major

                s3 = scp.tile([P, 82 * wmax], F16, tag="s3", name="s3", bufs=3)[
                    :, 0 : 82 * W
                ]
                Et = scp.tile([P, 80 * W], F16, tag="Et", name="Et")

                def do_premult():
                    nc.vector.tensor_tensor(
                        s3[:, 0 : 80 * W].rearrange("p (c j) -> p c j", c=80),
                        lg.rearrange("p (c j) -> p c j", c=80),
                        obj[:, None, :].broadcast_to([P, 80, W]),
                        ALU.mult,
                    )
                    nc.scalar.activation(Et[:], s3[:, 0 : 80 * W], ACTF.Exp)
                    if ones_pool:
                        nc.gpsimd.memset(s3[:, 81 * W : 82 * W], 1.0)
                    else:
                        nc.vector.memset(s3[:, 81 * W : 82 * W], 1.0)

                if premult_first:
                    do_premult()

                # ---- pairwise mask [P, G, W] (g-major; all operands step-1) ----
                # mask (A) lives until this window's matmuls finish -> own
                # triple-buffered tag; the other 4 tiles die at the mask op.
                A = pairp.tile([P, G * wmax], F16, tag="mk", name="mk", bufs=3)[
                    :, 0 : G * W
                ]
                scr = pairp.tile([P, 4 * G * wmax], F16, tag="scr", name="scr")
                B = scr[:, 0 * G * W : 1 * G * W]
                Cc = scr[:, 1 * G * W : 2 * G * W]
                D = scr[:, 2 * G * W : 3 * G * W]
                GP = scr[:, 3 * G * W : 4 * G * W]
                H = W // 2
                sh = lambda t: t[:].rearrange("p (g a t) -> p g a t", g=G, t=2)
                bc = lambda col: col.rearrange("p (a t) -> p a t", t=2)[
                    :, None
                ].broadcast_to([P, G, H, 2])
                gvq = lambda q: gv[:, q, :, None, :].broadcast_to([P, G, H, 2])
                eng_gpa = nc.gpsimd if gpa_pool else nc.vector
                eng_wy = nc.gpsimd if wy_pool else nc.vector
                eng_gpa.tensor_tensor(sh(GP), bc(pa3), gvq(4), ALU.add)
                nc.vector.tensor_tensor(sh(Cc), bc(py2), gvq(2), ALU.min)
                nc.vector.tensor_tensor(sh(D), bc(py1n), gvq(3), ALU.min)
                eng_wy.tensor_tensor(Cc[:], Cc[:], D[:], ALU.add)         # wy
                nc.vector.tensor_tensor(sh(A), bc(px2), gvq(0), ALU.min)
                nc.vector.tensor_tensor(sh(B), bc(px1n), gvq(1), ALU.min)
                nc.vector.tensor_tensor(A[:], A[:], B[:], ALU.add)        # wx
                nc.vector.tensor_scalar_max(A[:], A[:], 0.0)              # relu(wx)

                if not premult_first:
                    do_premult()

                # ---- LSE tree (in-place halves), split DVE/POOL per level ----
                # levels: (40, 20, 10, 5) halvings then 5 -> 2+2 -> 1+1 -> +1
                plan = [(40, 0), (20, 0), (10, 0), (5, 0)]
                tp = 99 if last else tree_pool_from
                for li, (half, _) in enumerate(plan):
                    eng = nc.gpsimd if li >= tp else nc.vector
                    eng.tensor_tensor(
                        Et[:, 0 : half * W],
                        Et[:, 0 : half * W],
                        Et[:, half * W : 2 * half * W],
                        ALU.add,
                    )
                eng = nc.gpsimd if 4 >= tp and not last else nc.vector
                eng.tensor_tensor(
                    Et[:, 0 : 2 * W], Et[:, 0 : 2 * W], Et[:, 2 * W : 4 * W], ALU.add
                )
                eng.tensor_tensor(Et[:, 0:W], Et[:, 0:W], Et[:, W : 2 * W], ALU.add)
                eng.tensor_tensor(Et[:, 0:W], Et[:, 0:W], Et[:, 4 * W : 5 * W], ALU.add)
                nc.scalar.activation(s3[:, 80 * W : 81 * W], Et[:, 0:W], ACTF.Ln)

                # ---- finish mask: V = relu(wx)*wy; mask = V >= GPa ----
                nc.vector.tensor_mul(A[:], A[:], Cc[:])                   # V
                nc.vector.tensor_tensor(A[:], A[:], GP[:], ALU.is_ge)     # mask

                # ---- PSUM-accumulated matmuls: T += mask^T @ [pm|LSE|1] ----
                mk3 = A[:].rearrange("p (g j) -> p g j", g=G)
                s33 = s3[:].rearrange("p (c j) -> p c j", c=82)
                for j in range(W):
                    r = r_base + j
                    nc.tensor.matmul(
                        psum_T[:],
                        mk3[:, :, j],
                        s33[:, :, j],
                        start=(r == 0),
                        stop=(r == R - 1),
                    )
                r_base += W

            out_t = constp.tile([G, 82], F32, name="out_t")
            nc.scalar.copy(out_t[:], psum_T[:])
            nc.sync.dma_start(res[:, :], out_t[:])
    return nc


def host_finish(res_list, label_batch):
    B = len(res_list)
    out = np.empty((1, B), np.float32)
    for b in range(B):
        T = res_list[b]
        cls = np.asarray(label_batch)[b, :, 0].astype(np.int32)
        S_T = T[np.arange(G), cls].sum()
        S_L = T[:, 80].sum()
        S_0 = T[:, 81].sum()
        out[0, b] = (S_L - S_T) / S_0
    return out


def prep_inputs(output, label_batch):
    B = output.shape[0]
    out32 = np.zeros((B, NPAD, 85), np.float32)
    out32[:, :N, :] = np.asarray(output, np.float32)
    x, y = out32[..., 0], out32[..., 1]
    w, h = out32[..., 2], out32[..., 3]
    planes = np.stack(
        [
            x + w / 2,          # px2
            w / 2 - x,          # -px1
            y + h / 2,          # py2
            h / 2 - y,          # -py1
            w * h / 3,          # pa/3
            out32[..., 4],      # obj
        ],
        axis=-1,
    )
    out16 = np.concatenate(
        [planes.astype(np.float16), out32[..., 5:].astype(np.float16)], axis=-1
    )  # [B, NPAD, 86]
    X = out16.reshape(B, R, P, ROW).transpose(0, 2, 3, 1)  # [B, P, 86, R]
    lb = np.asarray(label_batch, np.float32)
    maps = []
    for b in range(B):
        blocks = []
        rb = 0
        for W in WINDOWS:
            blocks.append(X[b][:, :, rb : rb + W].reshape(P, ROW * W))
            rb += W
        data = np.ascontiguousarray(np.concatenate(blocks, axis=1))
        g = lb[b]
        x1 = np.clip(g[:, 1] - g[:, 3] / 2, 0, 1) * SCALE
        x2 = np.clip(g[:, 1] + g[:, 3] / 2, 0, 1) * SCALE
        y1 = np.clip(g[:, 2] - g[:, 4] / 2, 0, 1) * SCALE
        y2 = np.clip(g[:, 2] + g[:, 4] / 2, 0, 1) * SCALE
        ga3 = (x2 - x1) * (y2 - y1) / 3.0
        pack = np.stack([x2, -x1, y2, -y1, ga3]).astype(np.float16)  # [5, G]
        gtrep = np.ascontiguousarray(
            np.repeat(pack[:, :, None], 2, axis=2)
        ).reshape(-1)
        maps.append({"data": data, "gtrep": gtrep})
    return maps


_CACHE = {}


def kernel(output, label_batch, prob_threshold):
    """Full inputs -> [1, B] loss. prob_threshold == 0 for this problem
    (keep = obj >= 0 always true; padded rows have w=h=0 so wx<0 -> unmatched)."""
    from concourse.bass_utils import run_bass_kernel_spmd

    output = np.asarray(output)
    label_batch = np.asarray(label_batch)
    B = output.shape[0]
    if "nc" not in _CACHE:
        _CACHE["nc"] = build_kernel_v2()
    nc = _CACHE["nc"]
    in_maps = prep_inputs(output, label_batch)
    r = run_bass_kernel_spmd(nc, in_maps, list(range(B)))
    res_list = [r.results[b]["res"] for b in range(B)]
    return host_finish(res_list, label_batch).astype(output.dtype)
